# revision 8
# baseline (speedup 1.0000x reference)
"""Embedding lookup on 8 Trainium2 NeuronCores.

Problem: x [16384, 4, 1] int32 indices into data [100000, 512] f32;
out[b, i, :] = data[x[b, i, 0], :].

Strategy (vocab/model-parallel via host routing):
  * Host sorts the 65536 flattened indices; core c serves sorted
    positions [c*8192, (c+1)*8192). Those rows lie in one contiguous
    table window (~12.7k rows), so each core receives only its window
    (~26 MiB) and window-relative indices.
  * Run compression: consecutive sorted positions whose rows increment
    by exactly +1 share one dynamic offset — the HW generic indirect
    DMA fetches a contiguous block of k rows per partition offset.
    Positions are split greedily into runs of length k <= 4, bucketed
    by k, each bucket padded to a multiple of 128 offsets (pad offsets
    fetch row 0 into throwaway device rows). ~45 indirect-DMA gathers
    of 128 offsets each per core instead of 64.
  * Device (raw Bass, GpSimd SWDGE): gathers issued back-to-back, each
    into its own SBUF tile; HWDGE stores chase them one-for-one with a
    dedicated completion semaphore per gather (16 SDMA engines can skew
    across ops, so per-op semaphores are required for exactness).
  * The device output is a known permutation of the final output; the
    host undoes it while unsharding (the "all-to-all on gathered
    rows" of the vocab-parallel scheme, folded into the host gather).

The kernel is rebuilt (and cached) per (window span, bucket op counts),
which are data-dependent; for a fixed input distribution this compiles
once.
"""

import numpy as np

import concourse.bacc as bacc
from concourse import bass, mybir
from concourse.bass_utils import run_bass_kernel_spmd

N_CORES = 8
VOCAB = 100000
DIM = 512
N_TOTAL = 16384 * 4
N_PER_CORE = N_TOTAL // N_CORES   # 8192
P = 128
K_MAX = 4

SPAN_STEP = 1024
_NC_CACHE = {}


def _build_nc(span, ops_per_k):
    # ops_per_k[k-1] = number of 128-offset gather ops with block length k
    n_cols = sum(ops_per_k)
    tot_rows = sum(ops * P * k for k, ops in enumerate(ops_per_k, start=1))

    nc = bacc.Bacc("TRN2", target_bir_lowering=False, debug=False)
    tab_t = nc.dram_tensor("table", [span, DIM], mybir.dt.float32, kind="ExternalInput")
    idx_t = nc.dram_tensor("idx32", [P, n_cols], mybir.dt.int32, kind="ExternalInput")
    out_t = nc.dram_tensor(
        "out", [tot_rows, DIM], mybir.dt.float32, kind="ExternalOutput"
    )

    # (k, idx column, DRAM row base) per op, in issue order
    ops = []
    col = 0
    base = 0
    for k, n_ops in enumerate(ops_per_k, start=1):
        for _ in range(n_ops):
            ops.append((k, col, base))
            col += 1
            base += P * k
    ops.reverse()  # big blocks first: frontload bytes, small-op tail
    n_ops_total = len(ops)

    with bass.ExitStack() as stack:
        enter = stack.enter_context
        idx_sb = enter(nc.sbuf_tensor("idx_sb", [P, n_cols], mybir.dt.int32))
        tiles = [
            enter(nc.sbuf_tensor(f"dst{i}", [P, k * DIM], mybir.dt.float32))
            for i, (k, _, _) in enumerate(ops)
        ]
        io = enter(nc.semaphore("io"))
        gsems = [enter(nc.semaphore(f"g{i}")) for i in range(n_ops_total)]
        ssem = enter(nc.semaphore("ssem"))
        block = enter(nc.Block())

        @block.gpsimd
        def _(gpsimd: bass.BassGpSimd):
            gpsimd.wait_ge(io, 16)  # idx32 in SBUF (loaded by sync engine)
            for i, (k, col_i, _) in enumerate(ops):
                gpsimd.indirect_dma_start(
                    out=tiles[i][:],
                    out_offset=None,
                    in_=tab_t[:],
                    in_offset=bass.IndirectOffsetOnAxis(
                        ap=idx_sb[:, col_i : col_i + 1], axis=0
                    ),
                ).then_inc(gsems[i], 16)
            for i in range(n_ops_total):
                gpsimd.wait_ge(gsems[i], 16)

        @block.sync
        def _(sync: bass.BassEngine):
            sync.dma_start(idx_sb[:], idx_t[:]).then_inc(io, 16)
            for i, (k, _, base_i) in enumerate(ops):
                sync.wait_ge(gsems[i], 16)
                sync.dma_start(
                    out_t[base_i : base_i + P * k].rearrange(
                        "(p m) d -> p (m d)", p=P
                    ),
                    tiles[i][:],
                ).then_inc(ssem, 16)
            sync.wait_ge(ssem, 16 * n_ops_total)

    nc.compile()
    return nc


def _get_nc(span, ops_per_k):
    key = (span, ops_per_k)
    if key not in _NC_CACHE:
        _NC_CACHE[key] = _build_nc(span, ops_per_k)
    return _NC_CACHE[key]


def _runs_of(rows):
    """Greedy split of sorted rows into +1-increment runs capped at K_MAX.
    Returns (run_pos, run_len): start position and length of each run."""
    n = len(rows)
    new_run = np.empty(n, dtype=bool)
    new_run[0] = True
    np.not_equal(np.diff(rows), 1, out=new_run[1:])
    nat_starts = np.flatnonzero(new_run)
    nat_lens = np.diff(np.r_[nat_starts, n])
    run_pos, run_len = [], []
    for s0, L in zip(nat_starts.tolist(), nat_lens.tolist()):
        nfull, rem = divmod(int(L), K_MAX)
        for m in range(nfull):
            run_pos.append(s0 + m * K_MAX)
            run_len.append(K_MAX)
        if rem:
            run_pos.append(s0 + nfull * K_MAX)
            run_len.append(rem)
    return np.asarray(run_pos), np.asarray(run_len)


def _shard(x, data):
    idx = np.asarray(x).reshape(-1).astype(np.int64)
    data = np.ascontiguousarray(np.asarray(data), dtype=np.float32)
    assert idx.shape == (N_TOTAL,), idx.shape
    assert data.shape == (VOCAB, DIM), data.shape

    order = np.argsort(idx, kind="stable")
    idx_sorted = idx[order]
    shards = idx_sorted.reshape(N_CORES, N_PER_CORE)
    los = shards[:, 0].copy()
    span_needed = int((shards[:, -1] - los).max()) + 1
    span = -(-span_needed // SPAN_STEP) * SPAN_STEP

    per_core = []
    counts = np.zeros((N_CORES, K_MAX), dtype=np.int64)
    for c in range(N_CORES):
        rel = (shards[c] - los[c]).astype(np.int32)
        run_pos, run_len = _runs_of(rel)
        per_core.append((rel, run_pos, run_len))
        for k in range(1, K_MAX + 1):
            counts[c, k - 1] = int((run_len == k).sum())
    ops_per_k = tuple(int(-(-counts[:, k - 1].max() // P)) for k in range(1, K_MAX + 1))

    n_cols = sum(ops_per_k)
    col_base = np.r_[0, np.cumsum(ops_per_k)][:K_MAX]
    row_base = np.r_[
        0, np.cumsum([o * P * k for k, o in enumerate(ops_per_k, start=1)])
    ][:K_MAX]
    tot_rows = sum(o * P * k for k, o in enumerate(ops_per_k, start=1))

    in_maps = []
    devrow = np.empty((N_CORES, N_PER_CORE), dtype=np.int64)
    for c in range(N_CORES):
        rel, run_pos, run_len = per_core[c]
        lo = int(los[c])
        tab = np.zeros((span, DIM), dtype=np.float32)
        avail = min(span, VOCAB - lo)
        tab[:avail] = data[lo : lo + avail]

        idx32 = np.zeros((P, n_cols), dtype=np.int32)  # pad offsets fetch row 0
        for k in range(1, K_MAX + 1):
            sel = run_len == k
            pos = run_pos[sel]          # start positions of k-runs
            starts = rel[pos]           # their start rows
            t = np.arange(len(pos))     # slot within bucket
            # slot t -> op j = t//P, partition p = t%P
            idx32[t % P, col_base[k - 1] + t // P] = starts
            # device rows: row_base + t*k + m  <-  position pos + m
            dr = row_base[k - 1] + t[:, None] * k + np.arange(k)[None, :]
            devrow[c, pos[:, None] + np.arange(k)[None, :]] = dr
        in_maps.append({"table": tab, "idx32": np.ascontiguousarray(idx32)})

    return in_maps, order, span, ops_per_k, devrow, tot_rows


def _run(x, data, **spmd_kwargs):
    x = np.asarray(x)
    in_maps, order, span, ops_per_k, devrow, tot_rows = _shard(x, data)
    nc = _get_nc(span, ops_per_k)
    res = run_bass_kernel_spmd(
        nc, in_maps, core_ids=list(range(N_CORES)), **spmd_kwargs
    )
    out = np.empty((N_TOTAL, DIM), dtype=np.float32)
    for c in range(N_CORES):
        dev = res.results[c]["out"].reshape(tot_rows, DIM)
        out[order[c * N_PER_CORE : (c + 1) * N_PER_CORE]] = dev[devrow[c]]
    return out.reshape(x.shape[:-1] + (DIM,)), res


def kernel(x, data):
    out, _ = _run(x, data)
    return out


# revision 9
# speedup vs baseline: 1.0369x; 1.0369x over previous
"""Embedding lookup on 8 Trainium2 NeuronCores.

Problem: x [16384, 4, 1] int32 indices into data [100000, 512] f32;
out[b, i, :] = data[x[b, i, 0], :].

Strategy (vocab/model-parallel via host routing):
  * Host sorts the 65536 flattened indices; core c serves sorted
    positions [c*8192, (c+1)*8192). Those rows lie in one contiguous
    table window (~12.7k rows), so each core receives only its window
    (~26 MiB) and window-relative indices.
  * Run compression: consecutive sorted positions whose rows increment
    by exactly +1 share one dynamic offset — the HW generic indirect
    DMA fetches a contiguous block of k rows per partition offset.
    Positions are split greedily into runs of length k <= 4, bucketed
    by k, each bucket padded to a multiple of 128 offsets (pad offsets
    fetch row 0 into throwaway device rows). ~45 indirect-DMA gathers
    of 128 offsets each per core instead of 64.
  * Device (raw Bass, GpSimd SWDGE): gathers issued back-to-back, each
    into its own SBUF tile; HWDGE stores chase them one-for-one with a
    dedicated completion semaphore per gather (16 SDMA engines can skew
    across ops, so per-op semaphores are required for exactness).
  * The device output is a known permutation of the final output; the
    host undoes it while unsharding (the "all-to-all on gathered
    rows" of the vocab-parallel scheme, folded into the host gather).

The kernel is rebuilt (and cached) per (window span, bucket op counts),
which are data-dependent; for a fixed input distribution this compiles
once.
"""

import numpy as np

import concourse.bacc as bacc
from concourse import bass, mybir
from concourse.bass_utils import run_bass_kernel_spmd

N_CORES = 8
VOCAB = 100000
DIM = 512
N_TOTAL = 16384 * 4
N_PER_CORE = N_TOTAL // N_CORES   # 8192
P = 128
K_MAX = 4

SPAN_STEP = 1024
_NC_CACHE = {}


def _build_nc(span, ops_per_k):
    # ops_per_k[k-1] = number of 128-offset gather ops with block length k
    n_cols = sum(ops_per_k)
    tot_rows = sum(ops * P * k for k, ops in enumerate(ops_per_k, start=1))

    nc = bacc.Bacc("TRN2", target_bir_lowering=False, debug=False)
    tab_t = nc.dram_tensor("table", [span, DIM], mybir.dt.float32, kind="ExternalInput")
    idx_t = nc.dram_tensor("idx32", [P, n_cols], mybir.dt.int32, kind="ExternalInput")
    out_t = nc.dram_tensor(
        "out", [tot_rows, DIM], mybir.dt.float32, kind="ExternalOutput"
    )

    # (k, idx column, DRAM row base) per op, in issue order
    ops = []
    col = 0
    base = 0
    for k, n_ops in enumerate(ops_per_k, start=1):
        for _ in range(n_ops):
            ops.append((k, col, base))
            col += 1
            base += P * k
    n_ops_total = len(ops)

    with bass.ExitStack() as stack:
        enter = stack.enter_context
        idx_sb = enter(nc.sbuf_tensor("idx_sb", [P, n_cols], mybir.dt.int32))
        tiles = [
            enter(nc.sbuf_tensor(f"dst{i}", [P, k * DIM], mybir.dt.float32))
            for i, (k, _, _) in enumerate(ops)
        ]
        io = enter(nc.semaphore("io"))
        gsems = [enter(nc.semaphore(f"g{i}")) for i in range(n_ops_total)]
        ssem = enter(nc.semaphore("ssem"))
        block = enter(nc.Block())

        @block.gpsimd
        def _(gpsimd: bass.BassGpSimd):
            gpsimd.wait_ge(io, 16)  # idx32 in SBUF (loaded by sync engine)
            for i, (k, col_i, _) in enumerate(ops):
                gpsimd.indirect_dma_start(
                    out=tiles[i][:],
                    out_offset=None,
                    in_=tab_t[:],
                    in_offset=bass.IndirectOffsetOnAxis(
                        ap=idx_sb[:, col_i : col_i + 1], axis=0
                    ),
                ).then_inc(gsems[i], 16)
            for i in range(n_ops_total):
                gpsimd.wait_ge(gsems[i], 16)

        @block.sync
        def _(sync: bass.BassEngine):
            sync.dma_start(idx_sb[:], idx_t[:]).then_inc(io, 16)
            for i, (k, _, base_i) in enumerate(ops):
                sync.wait_ge(gsems[i], 16)
                sync.dma_start(
                    out_t[base_i : base_i + P * k].rearrange(
                        "(p m) d -> p (m d)", p=P
                    ),
                    tiles[i][:],
                ).then_inc(ssem, 16)
            sync.wait_ge(ssem, 16 * n_ops_total)

    nc.compile()
    return nc


def _get_nc(span, ops_per_k):
    key = (span, ops_per_k)
    if key not in _NC_CACHE:
        _NC_CACHE[key] = _build_nc(span, ops_per_k)
    return _NC_CACHE[key]


def _runs_of(rows):
    """Greedy split of sorted rows into +1-increment runs capped at K_MAX.
    Returns (run_pos, run_len): start position and length of each run."""
    n = len(rows)
    new_run = np.empty(n, dtype=bool)
    new_run[0] = True
    np.not_equal(np.diff(rows), 1, out=new_run[1:])
    nat_starts = np.flatnonzero(new_run)
    nat_lens = np.diff(np.r_[nat_starts, n])
    run_pos, run_len = [], []
    for s0, L in zip(nat_starts.tolist(), nat_lens.tolist()):
        nfull, rem = divmod(int(L), K_MAX)
        for m in range(nfull):
            run_pos.append(s0 + m * K_MAX)
            run_len.append(K_MAX)
        if rem:
            run_pos.append(s0 + nfull * K_MAX)
            run_len.append(rem)
    return np.asarray(run_pos), np.asarray(run_len)


def _shard(x, data):
    idx = np.asarray(x).reshape(-1).astype(np.int64)
    data = np.ascontiguousarray(np.asarray(data), dtype=np.float32)
    assert idx.shape == (N_TOTAL,), idx.shape
    assert data.shape == (VOCAB, DIM), data.shape

    order = np.argsort(idx, kind="stable")
    idx_sorted = idx[order]
    shards = idx_sorted.reshape(N_CORES, N_PER_CORE)
    los = shards[:, 0].copy()
    span_needed = int((shards[:, -1] - los).max()) + 1
    span = -(-span_needed // SPAN_STEP) * SPAN_STEP

    per_core = []
    counts = np.zeros((N_CORES, K_MAX), dtype=np.int64)
    for c in range(N_CORES):
        rel = (shards[c] - los[c]).astype(np.int32)
        run_pos, run_len = _runs_of(rel)
        per_core.append((rel, run_pos, run_len))
        for k in range(1, K_MAX + 1):
            counts[c, k - 1] = int((run_len == k).sum())
    ops_per_k = tuple(int(-(-counts[:, k - 1].max() // P)) for k in range(1, K_MAX + 1))

    n_cols = sum(ops_per_k)
    col_base = np.r_[0, np.cumsum(ops_per_k)][:K_MAX]
    row_base = np.r_[
        0, np.cumsum([o * P * k for k, o in enumerate(ops_per_k, start=1)])
    ][:K_MAX]
    tot_rows = sum(o * P * k for k, o in enumerate(ops_per_k, start=1))

    in_maps = []
    devrow = np.empty((N_CORES, N_PER_CORE), dtype=np.int64)
    for c in range(N_CORES):
        rel, run_pos, run_len = per_core[c]
        lo = int(los[c])
        tab = np.zeros((span, DIM), dtype=np.float32)
        avail = min(span, VOCAB - lo)
        tab[:avail] = data[lo : lo + avail]

        idx32 = np.zeros((P, n_cols), dtype=np.int32)  # pad offsets fetch row 0
        for k in range(1, K_MAX + 1):
            sel = run_len == k
            pos = run_pos[sel]          # start positions of k-runs
            starts = rel[pos]           # their start rows
            t = np.arange(len(pos))     # slot within bucket
            # slot t -> op j = t//P, partition p = t%P
            idx32[t % P, col_base[k - 1] + t // P] = starts
            # device rows: row_base + t*k + m  <-  position pos + m
            dr = row_base[k - 1] + t[:, None] * k + np.arange(k)[None, :]
            devrow[c, pos[:, None] + np.arange(k)[None, :]] = dr
        in_maps.append({"table": tab, "idx32": np.ascontiguousarray(idx32)})

    return in_maps, order, span, ops_per_k, devrow, tot_rows


def _run(x, data, **spmd_kwargs):
    x = np.asarray(x)
    in_maps, order, span, ops_per_k, devrow, tot_rows = _shard(x, data)
    nc = _get_nc(span, ops_per_k)
    res = run_bass_kernel_spmd(
        nc, in_maps, core_ids=list(range(N_CORES)), **spmd_kwargs
    )
    out = np.empty((N_TOTAL, DIM), dtype=np.float32)
    for c in range(N_CORES):
        dev = res.results[c]["out"].reshape(tot_rows, DIM)
        out[order[c * N_PER_CORE : (c + 1) * N_PER_CORE]] = dev[devrow[c]]
    return out.reshape(x.shape[:-1] + (DIM,)), res


def kernel(x, data):
    out, _ = _run(x, data)
    return out
